# revision 11
# baseline (speedup 1.0000x reference)
"""Chamfer distance kernel for Trainium2 (8 NeuronCores).

Problem: points1 [4,8192,3], points2 [4,8192,3] f32 ->
  scalar = sum_b [ sum_n min_m d2(b,n,m) + sum_m min_n d2(b,n,m) ]

Strategy (v2: single-computation)
--------------------------------
Core c handles batch b=c//2, row-half h=c%2: rows = p1[b, h*4096:...],
cols = all of p2[b].  Each distance tile is computed ONCE; both the
row-direction mins (dist1 half) and column-direction partial mins
(dist2 partial, finished on the host by min-ing the two half-cores)
are extracted from it:

    PE  : d2 = ||x||^2 - 2 x.y + ||y||^2 via K=24 augmented bf16 matmul
          (3-way bf16 splits keep PSUM fp32 accurate to ~2^-24)
    ACT : drains PSUM -> SBUF bf16 (1 elem/lane/cyc @1.2GHz)
    DVE : colmin running tensor_tensor(min) on bf16 (2x mode)
          rowmin tensor_tensor_reduce(min,min) over the tile's two
          column halves, accumulator chained through `scalar` (2x mode)

ACT (218us) and DVE (205us) run concurrently; PE (~30us) is far ahead.
Column partials leave via PE transpose + tensor_reduce; the host mins
the two half-cores' column vectors and sums.
"""

import os

import numpy as np

import concourse.bacc as bacc
import concourse.mybir as mybir
from concourse.bass_utils import run_bass_kernel_spmd
from concourse.masks import make_identity
from concourse.tile import TileContext

FP32 = mybir.dt.float32
BF16 = mybir.dt.bfloat16
MIN = mybir.AluOpType.min
ADD = mybir.AluOpType.add
MULT = mybir.AluOpType.mult
SUB = mybir.AluOpType.subtract

B, N, D = 4, 8192, 3
NCORES = 8
ROWS = N // 2          # rows per core (4096)
RT = ROWS // 128       # row tiles (32)
CG = N // 128          # column groups of 128 points (64)

# k-row layout inside each 32-partition strip (K=24):
#  j 0-2  : xh * (-2 yh)     j 9-11 : xh * (-2 ym)    j 18-20 : 1 * ynrm_{h,m,l}
#  j 3-5  : xm * (-2 yh)     j 12-14: xm * (-2 ym)    j 21-23 : xnrm_{h,m,l} * 1
#  j 6-8  : xl * (-2 yh)     j 15-17: xh * (-2 yl)
X_OFFS = {"hi": (0, 9, 15), "mid": (3, 12), "lo": (6,)}
Y_OFFS = {"hi": (0, 3, 6), "mid": (9, 12), "lo": (15,)}
KROWS = 24

_CACHE = {}


def _split3(nc, pool, nat, ncols, tag):
    """3-way bf16 split of an f32 tile: nat ~= hi + mid + lo exactly enough."""
    hi = pool.tile([128, ncols], BF16, tag=f"{tag}_hi", name=f"{tag}_hi")
    r1 = pool.tile([128, ncols], FP32, tag=f"{tag}_r1", name=f"{tag}_r1")
    mid = pool.tile([128, ncols], BF16, tag=f"{tag}_mid", name=f"{tag}_mid")
    r2 = pool.tile([128, ncols], FP32, tag=f"{tag}_r2", name=f"{tag}_r2")
    lo = pool.tile([128, ncols], BF16, tag=f"{tag}_lo", name=f"{tag}_lo")
    nc.vector.tensor_copy(hi[:], nat[:])
    nc.vector.tensor_tensor(r1[:], nat[:], hi[:], SUB)
    nc.vector.tensor_copy(mid[:], r1[:])
    nc.vector.tensor_tensor(r2[:], r1[:], mid[:], SUB)
    nc.vector.tensor_copy(lo[:], r2[:])
    return {"hi": hi, "mid": mid, "lo": lo}


def _norm_split(nc, lt, nat, ngroups, tag):
    """||p||^2 per point (f32, exact) then 3-way bf16 split -> [128, ng, 3]."""
    sq = lt.tile([128, 3 * ngroups], FP32, tag=f"sq{tag}", name=f"sq{tag}")
    nc.vector.tensor_tensor(sq[:], nat[:], nat[:], MULT)
    sqv = sq.rearrange("p (c d) -> p c d", d=3)
    nrm_a = lt.tile([128, ngroups], FP32, tag=f"nrma{tag}", name=f"nrma{tag}")
    nc.vector.tensor_tensor(nrm_a[:], sqv[:, :, 0], sqv[:, :, 1], ADD)
    nrm = lt.tile([128, ngroups], FP32, tag=f"nrm{tag}", name=f"nrm{tag}")
    nc.vector.tensor_tensor(nrm[:], nrm_a[:], sqv[:, :, 2], ADD)
    nrms = lt.tile([128, 3 * ngroups], BF16, tag=f"nrms{tag}", name=f"nrms{tag}")
    nv = nrms.rearrange("p (c s) -> p c s", s=3)
    rn1 = lt.tile([128, ngroups], FP32, tag=f"rn1_{tag}", name=f"rn1_{tag}")
    rn2 = lt.tile([128, ngroups], FP32, tag=f"rn2_{tag}", name=f"rn2_{tag}")
    nc.vector.tensor_copy(nv[:, :, 0], nrm[:])
    nc.vector.tensor_tensor(rn1[:], nrm[:], nv[:, :, 0], SUB)
    nc.vector.tensor_copy(nv[:, :, 1], rn1[:])
    nc.vector.tensor_tensor(rn2[:], rn1[:], nv[:, :, 1], SUB)
    nc.vector.tensor_copy(nv[:, :, 2], rn2[:])
    return nv


def _emit(nc, tc, p1h, p2, out_dram, outcol_dram, stack):
    lt = stack.enter_context(tc.tile_pool(name="lt", bufs=1))

    ident = lt.tile([128, 128], BF16, tag="ident", name="ident")
    make_identity(nc, ident[:])

    # ---- natural-layout loads ------------------------------------------
    nat_r = lt.tile([128, 3 * RT], FP32, tag="nat_r", name="nat_r")
    nc.sync.dma_start(
        out=nat_r[:], in_=p1h.rearrange("(p c) d -> p (c d)", p=128)
    )
    nat_c = lt.tile([128, 3 * CG], FP32, tag="nat_c", name="nat_c")
    nc.sync.dma_start(
        out=nat_c[:], in_=p2.rearrange("(p c) d -> p (c d)", p=128)
    )

    # ---- splits, norms, aug staging tiles ------------------------------
    xs = _split3(nc, lt, nat_r, 3 * RT, "x")
    ys = _split3(nc, lt, nat_c, 3 * CG, "y")
    nv = _norm_split(nc, lt, nat_c, CG, "y")     # ||y||^2 splits
    nvr = _norm_split(nc, lt, nat_r, RT, "x")    # ||x||^2 splits

    # cols side (rhs) staging: [p, c, g, j]
    bt = lt.tile([128, 128 * CG], BF16, tag="bt", name="bt")
    btv = bt.rearrange("p (c g j) -> p c g j", g=4, j=32)
    for g in range(4):
        nc.gpsimd.memset(btv[:, :, g, KROWS:32], 0.0)
    for g in range(4):
        for part, offs in Y_OFFS.items():
            src = ys[part].rearrange("p (c d) -> p c d", d=3)
            for off in offs:
                nc.vector.tensor_scalar(
                    btv[:, :, g, off : off + 3], src, -2.0, None, MULT
                )
        nc.vector.tensor_copy(btv[:, :, g, 18:21], nv[:, :, :])
        nc.vector.memset(btv[:, :, g, 21:24], 1.0)

    # rows side (weights) staging: [p, r, g, j]
    wt = lt.tile([128, 128 * RT], BF16, tag="wt", name="wt")
    wtv = wt.rearrange("p (r g j) -> p r g j", g=4, j=32)
    for g in range(4):
        nc.gpsimd.memset(wtv[:, :, g, KROWS:32], 0.0)
    for g in range(4):
        for part, offs in X_OFFS.items():
            src = xs[part].rearrange("p (c d) -> p c d", d=3)
            for off in offs:
                nc.vector.tensor_copy(wtv[:, :, g, off : off + 3], src)
        nc.vector.memset(wtv[:, :, g, 18:21], 1.0)
        nc.vector.tensor_copy(wtv[:, :, g, 21:24], nvr[:, :, :])

    aug2 = lt.tile([128, 128 * CG], BF16, tag="aug2", name="aug2")
    wsb = lt.tile([128, 128 * RT], BF16, tag="wsb", name="wsb")

    # ---- PE transposes into strip layout (phase A psum pool, scoped) ---
    with tc.tile_pool(name="tp", bufs=2, space="PSUM") as tp:
        for t in range(CG // 4):
            pt = tp.tile([128, 512], BF16, tag="tp", name="tp")
            for q in range(4):
                c = 4 * t + q
                nc.tensor.transpose(
                    pt[:, 128 * q : 128 * q + 128],
                    bt[:, 128 * c : 128 * c + 128],
                    ident[:],
                )
            nc.scalar.copy(aug2[:, 512 * t : 512 * t + 512], pt[:])
        for t in range(RT // 4):
            pt = tp.tile([128, 512], BF16, tag="tp", name="tp")
            for q in range(4):
                r = 4 * t + q
                nc.tensor.transpose(
                    pt[:, 128 * q : 128 * q + 128],
                    wt[:, 128 * r : 128 * r + 128],
                    ident[:],
                )
            nc.scalar.copy(wsb[:, 512 * t : 512 * t + 512], pt[:])

    # ---- main distance loop --------------------------------------------
    # colrun[p, 128*c + pp] accumulates min over row tiles; rowmins[:, r]
    # gets the rowmin of row tile r via a chained tensor_tensor_reduce.
    gp_k = int(os.environ.get("CHAMFER_GPK", "0"))  # of RT tiles, f1 on GpSimd
    colrun = lt.tile([128, 128 * CG], BF16, tag="colrun", name="colrun")
    rowmins = lt.tile([128, RT], FP32, tag="rowmins", name="rowmins")

    with (
        tc.tile_pool(name="mp", bufs=2, space="PSUM") as mp,
        tc.tile_pool(name="ch", bufs=3) as ch,
    ):
        for r in range(RT):
            sc = ch.tile([128, 8192], BF16, tag="sc", name="sc")
            for t in range(4):
                pk = mp.tile([128, 2048], FP32, tag="pk", name="pk")
                for q in range(4):
                    c = 4 * t + q
                    nc.tensor.matmul(
                        pk[:, 512 * q : 512 * q + 512],
                        wsb[32 * q : 32 * q + KROWS, 128 * r : 128 * r + 128],
                        aug2[32 * q : 32 * q + KROWS, 512 * c : 512 * c + 512],
                        start=True,
                        stop=True,
                        tile_position=(32 * q, 0),
                    )
                # ACT drains PSUM -> SBUF bf16
                nc.scalar.copy(sc[:, 2048 * t : 2048 * t + 2048], pk[:])
            # DVE: running column-min (bf16 2x), one wide op
            if r == 0:
                nc.vector.tensor_copy(colrun[:], sc[:])
            else:
                nc.vector.tensor_tensor(colrun[:], sc[:], colrun[:], MIN)
            # row-min via pairwise-min fold cascade (bf16 2x on DVE);
            # the first (largest) fold runs on the otherwise-idle GpSimd
            f1 = ch.tile([128, 4096], BF16, tag="f1", name="f1")
            gp_take = ((r + 1) * gp_k) // RT - (r * gp_k) // RT == 1
            f1_eng = nc.gpsimd if gp_take else nc.vector
            f1_eng.tensor_tensor(f1[:], sc[:, 0:4096], sc[:, 4096:8192], MIN)
            f2 = ch.tile([128, 2048], BF16, tag="f2", name="f2")
            nc.vector.tensor_tensor(f2[:], f1[:, 0:2048], f1[:, 2048:4096], MIN)
            f3 = ch.tile([128, 1024], BF16, tag="f3", name="f3")
            nc.vector.tensor_tensor(f3[:], f2[:, 0:1024], f2[:, 1024:2048], MIN)
            f4 = ch.tile([128, 512], BF16, tag="f4", name="f4")
            nc.vector.tensor_tensor(f4[:], f3[:, 0:512], f3[:, 512:1024], MIN)
            f5 = ch.tile([128, 256], BF16, tag="f5", name="f5")
            nc.vector.tensor_tensor(f5[:], f4[:, 0:256], f4[:, 256:512], MIN)
            nc.vector.tensor_reduce(
                out=rowmins[:, r : r + 1],
                in_=f5[:],
                op=MIN,
                axis=mybir.AxisListType.X,
            )

    # ---- epilogue -------------------------------------------------------
    # rowmin sum -> [128,1] -> cross-partition sum via K=128 matmul
    racc1 = lt.tile([128, 1], FP32, tag="racc1", name="racc1")
    nc.vector.reduce_sum(out=racc1[:], in_=rowmins[:], axis=mybir.AxisListType.X)
    ones = lt.tile([128, 1], FP32, tag="ones", name="ones")
    nc.vector.memset(ones[:], 1.0)
    out_sb = lt.tile([1, 1], FP32, tag="out_sb", name="out_sb")

    # column partial mins: transpose colrun 128x128 blocks, reduce free axis
    colfin = lt.tile([128, CG], FP32, tag="colfin", name="colfin")
    with tc.tile_pool(name="fp", bufs=2, space="PSUM") as fp:
        for t in range(CG // 4):
            ct = fp.tile([128, 512], BF16, tag="ct", name="ct")
            for q in range(4):
                gblk = 4 * t + q
                nc.tensor.transpose(
                    ct[:, 128 * q : 128 * q + 128],
                    colrun[:, 128 * gblk : 128 * gblk + 128],
                    ident[:],
                )
            nc.vector.tensor_reduce(
                out=colfin[:, 4 * t : 4 * t + 4],
                in_=ct.rearrange("p (g x) -> p g x", x=128),
                op=MIN,
                axis=mybir.AxisListType.X,
            )
        fin = fp.tile([1, 1], FP32, tag="fin", name="fin")
        nc.tensor.matmul(fin[:], racc1[:], ones[:], start=True, stop=True)
        nc.vector.tensor_copy(out_sb[:], fin[:])

    nc.sync.dma_start(out=out_dram, in_=out_sb[:])
    nc.sync.dma_start(out=outcol_dram, in_=colfin[:])


def _build():
    if "nc" in _CACHE:
        return _CACHE["nc"]
    nc = bacc.Bacc("TRN2", target_bir_lowering=False, debug=False, num_devices=NCORES)
    p1h = nc.dram_tensor("p1h", [ROWS, D], FP32, kind="ExternalInput").ap()
    p2 = nc.dram_tensor("p2", [N, D], FP32, kind="ExternalInput").ap()
    out = nc.dram_tensor("out", [1, 1], FP32, kind="ExternalOutput").ap()
    outcol = nc.dram_tensor("outcol", [128, CG], FP32, kind="ExternalOutput").ap()
    from contextlib import ExitStack

    with TileContext(nc) as tc:
        with ExitStack() as stack:
            _emit(nc, tc, p1h, p2, out, outcol, stack)
    nc.compile()
    _CACHE["nc"] = nc
    return nc


LAST_RESULT = None


def kernel(points1: np.ndarray, points2: np.ndarray) -> np.ndarray:
    global LAST_RESULT
    nc = _build()
    points1 = np.ascontiguousarray(np.asarray(points1, dtype=np.float32))
    points2 = np.ascontiguousarray(np.asarray(points2, dtype=np.float32))
    in_maps = []
    for c in range(NCORES):
        b, h = c // 2, c % 2
        in_maps.append(
            {
                "p1h": np.ascontiguousarray(points1[b, h * ROWS : (h + 1) * ROWS]),
                "p2": points2[b],
            }
        )
    trace = bool(int(os.environ.get("CHAMFER_TRACE", "0")))
    res = run_bass_kernel_spmd(nc, in_maps, core_ids=list(range(NCORES)), trace=trace)
    LAST_RESULT = res
    total = np.float64(0.0)
    for b in range(B):
        ra, rb = res.results[2 * b], res.results[2 * b + 1]
        total += np.float64(ra["out"][0, 0]) + np.float64(rb["out"][0, 0])
        colmin = np.minimum(
            ra["outcol"].astype(np.float64), rb["outcol"].astype(np.float64)
        )
        total += colmin.sum()
    return np.float32(total)
